# revision 1
# baseline (speedup 1.0000x reference)
"""CML int8-sim Trainium2 kernel.

Strategy (data-parallel over batch, 8 cores, B=256 -> 32 rows/core):
The per-step update
    mapped = r*g*(1-g)
    coupled = 0.5*(circ_conv(mapped, K) + mapped @ W_cc)
    g' = (1-beta)*((1-eps)*mapped + eps*coupled) + beta*drive
    g  = round(127*g')/127
is linear in `mapped` after the logistic map, so the circular conv, the
global coupling, the eps/beta site scalings, the diagonal passthrough
and the 127 quantization scale all fold into one precomputed matrix:
    127*g' = mapped @ W3_127 + 127*beta*drive
Each step on-device is then: elementwise logistic map (DVE), a PE
transpose to feed the batch-stationary matmul, a 4-way column-tiled
fp32 matmul [32,2048]x[2048,2048], and a fused add+round (DVE, exact
round-to-nearest-even via the 1.5*2^23 magic constant).

State is kept 127-scaled ("v = 127*g", integer-valued fp32), in a
scrambled [128, 512] layout (partition = 32*ntile + batch) so that all
elementwise work uses the full 128 partitions.

The default "fp32p" mode additionally splits every per-step tensor into
column halves and orders the k-rounds block-major, so the DVE/ACT tail
of one half overlaps the PE matmuls of the other and the next step's
matmuls start as soon as the transposed block they need is ready
(~1.6x over the unpipelined version; ~16-20us/step, ~1.0-1.3ms total).
"""
import numpy as np

B, N, NCORES = 256, 2048, 8
BL = B // NCORES          # 32 batch rows per core
MAGIC = 12582912.0        # 1.5*2^23: (x+M)-M == RNE round for |x| < 2^22

MODE = "fp32p"            # "fp32p" (pipelined, default) | "fp32" | "fp16x3"
_programs = {}
_last_in_maps = None


LOSCALE = 64.0  # 2^6: keeps fp16 lo-parts out of subnormal range


def _build_program(steps, kc_rounds=16, nsplit=512, mode="fp32", wbufs=2):
    import concourse.mybir as mybir
    import concourse.tile as tile
    from concourse import bacc

    f32 = mybir.dt.float32
    f16 = mybir.dt.float16
    sub = mybir.AluOpType.subtract
    add = mybir.AluOpType.add
    mult = mybir.AluOpType.mult

    nc = bacc.Bacc("TRN2", target_bir_lowering=False, debug=False)

    if mode != "fp16x3":
        d_Wp = nc.dram_tensor("Wp", [128, 16 * N], f32, kind="ExternalInput").ap()
    else:
        d_Wh = nc.dram_tensor("Wh", [128, 16 * N], f16, kind="ExternalInput").ap()
        d_Wl = nc.dram_tensor("Wl", [128, 16 * N], f16, kind="ExternalInput").ap()
    d_g0 = nc.dram_tensor("g0_bn", [128, 512], f32, kind="ExternalInput").ap()
    d_C = nc.dram_tensor("C_bn", [128, 512], f32, kind="ExternalInput").ap()
    d_R1 = nc.dram_tensor("R1_bn", [128, 512], f32, kind="ExternalInput").ap()
    d_R2 = nc.dram_tensor("R2_bn", [128, 512], f32, kind="ExternalInput").ap()
    d_id = nc.dram_tensor("ident", [128, 128], f32, kind="ExternalInput").ap()
    d_out = nc.dram_tensor("v_out", [128, 512], f32, kind="ExternalOutput").ap()

    with tile.TileContext(nc) as tc:
        with tc.tile_pool(name="consts", bufs=1) as cp, \
             tc.tile_pool(name="work", bufs=wbufs) as wp, \
             tc.tile_pool(name="psum", bufs=2, space="PSUM") as pp:
            if mode != "fp16x3":
                s_Wp = cp.tile([128, 16 * N], f32)
            else:
                s_Wh = cp.tile([128, 16 * N], f16)
                s_Wl = cp.tile([128, 16 * N], f16)
            s_g0 = cp.tile([128, 512], f32)
            s_C = cp.tile([128, 512], f32)
            s_R1 = cp.tile([128, 512], f32)
            s_R2 = cp.tile([128, 512], f32)
            s_id = cp.tile([128, 128], f32)
            nc.sync.dma_start(out=s_g0[:], in_=d_g0[:])
            nc.sync.dma_start(out=s_C[:], in_=d_C[:])
            nc.sync.dma_start(out=s_R1[:], in_=d_R1[:])
            nc.sync.dma_start(out=s_R2[:], in_=d_R2[:])
            nc.sync.dma_start(out=s_id[:], in_=d_id[:])
            # W3 is big: DMA per 128-chunk so first matmuls can start
            # before the whole matrix has landed.
            for kc in range(16):
                if mode != "fp16x3":
                    nc.sync.dma_start(out=s_Wp[:, N * kc:N * (kc + 1)],
                                      in_=d_Wp[:, N * kc:N * (kc + 1)])
                else:
                    nc.sync.dma_start(out=s_Wh[:, N * kc:N * (kc + 1)],
                                      in_=d_Wh[:, N * kc:N * (kc + 1)])
                    nc.sync.dma_start(out=s_Wl[:, N * kc:N * (kc + 1)],
                                      in_=d_Wl[:, N * kc:N * (kc + 1)])

            if mode == "fp32p":
                # Pipelined: everything split into column halves so the
                # add/round/logistic/transpose tail of half 0 overlaps
                # the PE matmuls of half 1, and the next step's matmuls
                # (block-major k order) start as soon as the transposed
                # block they need is ready.
                ORDER = [0, 4, 8, 12, 1, 5, 9, 13, 2, 6, 10, 14, 3, 7, 11, 15]
                vh_prev = None
                for t in range(steps):
                    mts = []
                    for h in (0, 1):
                        if t == 0:
                            src = s_g0[:, 256 * h:256 * (h + 1)]
                            Rt, shift = s_R1, 1.0
                        else:
                            src = vh_prev[h][:]
                            Rt, shift = s_R2, 127.0
                        a = wp.tile([128, 256], f32, tag=f"a{h}")
                        nc.vector.tensor_mul(
                            out=a[:], in0=Rt[:, 256 * h:256 * (h + 1)], in1=src)
                        mneg = wp.tile([128, 256], f32, tag=f"mneg{h}")
                        nc.vector.scalar_tensor_tensor(
                            out=mneg[:], in0=src, scalar=shift, in1=a[:],
                            op0=sub, op1=mult)
                        pT = pp.tile([128, 256], f32, tag=f"pT{h}")
                        for b in range(2):
                            nc.tensor.transpose(
                                pT[:, 128 * b:128 * (b + 1)],
                                mneg[:, 128 * b:128 * (b + 1)], s_id[:])
                        mt = wp.tile([128, 256], f32, tag=f"mTs{h}")
                        nc.scalar.copy(mt[:], pT[:])
                        mts.append(mt)

                    def lhs(kc):
                        bk = kc % 4
                        off = 128 * (bk % 2) + 32 * (kc // 4)
                        return mts[bk // 2][:, off:off + 32]

                    vh = []
                    for h in (0, 1):
                        P1 = pp.tile([128, 256], f32, tag=f"P1h{h}")
                        for idx, kc in enumerate(ORDER):
                            lh = lhs(kc)
                            for j in range(4):
                                base = N * kc + 512 * j + 256 * h
                                nc.tensor.matmul(
                                    out=P1[32 * j:32 * (j + 1), :],
                                    lhsT=lh, rhs=s_Wp[:, base:base + 256],
                                    start=(idx == 0), stop=(idx == 15),
                                    tile_position=(0, 32 * j))
                        tmp = wp.tile([128, 256], f32, tag=f"tmp{h}")
                        nc.vector.tensor_add(
                            out=tmp[:], in0=P1[:],
                            in1=s_C[:, 256 * h:256 * (h + 1)])
                        v = wp.tile([128, 256], f32, tag=f"v{h}")
                        nc.vector.tensor_scalar(
                            out=v[:], in0=tmp[:], scalar1=MAGIC, scalar2=MAGIC,
                            op0=add, op1=sub)
                        vh.append(v)
                    vh_prev = vh

                nc.sync.dma_start(out=d_out[:, 0:256], in_=vh_prev[0][:])
                nc.sync.dma_start(out=d_out[:, 256:512], in_=vh_prev[1][:])

            v_prev = None
            for t in range(steps if mode != "fp32p" else 0):
                if t == 0:
                    src, Rt, shift = s_g0, s_R1, 1.0
                else:
                    src, Rt, shift = v_prev, s_R2, 127.0
                # mneg = (src - shift) * (Rt .* src)  == -mapped
                a = wp.tile([128, 512], f32, tag="a")
                nc.vector.tensor_mul(out=a[:], in0=Rt[:], in1=src[:])
                mneg = wp.tile([128, 512], f32, tag="mneg")
                nc.vector.scalar_tensor_tensor(
                    out=mneg[:], in0=src[:], scalar=shift, in1=a[:],
                    op0=sub, op1=mult)
                # transpose mneg into lhsT ordering (4 x [128,128] blocks)
                pT = pp.tile([128, 512], f32, tag="pT")
                for bk in range(4):
                    nc.tensor.transpose(
                        pT[:, 128 * bk:128 * (bk + 1)],
                        mneg[:, 128 * bk:128 * (bk + 1)], s_id[:])
                mTs = wp.tile([128, 512], f32, tag="mTs")
                nc.scalar.copy(mTs[:], pT[:])

                def lsl(kc):
                    o = 128 * (kc % 4) + 32 * (kc // 4)
                    return o, o + 32

                if mode == "fp32":
                    P1 = pp.tile([128, 512], f32, tag="P1")
                    for kc in range(kc_rounds):
                        o0, o1 = lsl(kc)
                        lh = mTs[:, o0:o1]
                        for j in range(4):
                            for h in range(512 // nsplit):
                                nc.tensor.matmul(
                                    out=P1[32 * j:32 * (j + 1),
                                           nsplit * h:nsplit * (h + 1)],
                                    lhsT=lh,
                                    rhs=s_Wp[:, N * kc + 512 * j + nsplit * h:
                                             N * kc + 512 * j + nsplit * (h + 1)],
                                    start=(kc == 0), stop=(kc == kc_rounds - 1),
                                    tile_position=(0, 32 * j))
                    tmp = wp.tile([128, 512], f32, tag="tmp")
                    nc.vector.tensor_add(out=tmp[:], in0=P1[:], in1=s_C[:])
                else:
                    # split transposed mapped into hi + scaled lo (fp16)
                    mh = wp.tile([128, 512], f16, tag="mh")
                    nc.vector.tensor_copy(out=mh[:], in_=mTs[:])
                    mlr = wp.tile([128, 512], f32, tag="mlr")
                    nc.vector.tensor_sub(out=mlr[:], in0=mTs[:], in1=mh[:])
                    ml2 = wp.tile([128, 512], f16, tag="ml2")
                    nc.vector.tensor_scalar(
                        out=ml2[:], in0=mlr[:], scalar1=LOSCALE, scalar2=None,
                        op0=mult)
                    P1 = pp.tile([128, 512], f32, tag="P1")
                    P2 = pp.tile([128, 512], f32, tag="P2")
                    for kc in range(kc_rounds):
                        o0, o1 = lsl(kc)
                        for j in range(4):
                            rh = s_Wh[:, N * kc + 512 * j:N * kc + 512 * (j + 1)]
                            nc.tensor.matmul(
                                out=P1[32 * j:32 * (j + 1), :],
                                lhsT=mh[:, o0:o1], rhs=rh,
                                start=(kc == 0), stop=(kc == kc_rounds - 1),
                                tile_position=(0, 32 * j))
                        for j in range(4):
                            rh = s_Wh[:, N * kc + 512 * j:N * kc + 512 * (j + 1)]
                            nc.tensor.matmul(
                                out=P2[32 * j:32 * (j + 1), :],
                                lhsT=ml2[:, o0:o1], rhs=rh,
                                start=(kc == 0), stop=False,
                                tile_position=(0, 32 * j))
                        for j in range(4):
                            rl = s_Wl[:, N * kc + 512 * j:N * kc + 512 * (j + 1)]
                            nc.tensor.matmul(
                                out=P2[32 * j:32 * (j + 1), :],
                                lhsT=mh[:, o0:o1], rhs=rl,
                                start=False, stop=(kc == kc_rounds - 1),
                                tile_position=(0, 32 * j))
                    # tmp = (P2/LOSCALE + C) + P1   (one PSUM operand per op)
                    tmp1 = wp.tile([128, 512], f32, tag="tmp1")
                    nc.vector.scalar_tensor_tensor(
                        out=tmp1[:], in0=P2[:], scalar=1.0 / LOSCALE,
                        in1=s_C[:], op0=mult, op1=add)
                    tmp = wp.tile([128, 512], f32, tag="tmp")
                    nc.vector.tensor_add(out=tmp[:], in0=tmp1[:], in1=P1[:])

                # v = rne_round(tmp)
                v = wp.tile([128, 512], f32, tag="v")
                nc.vector.tensor_scalar(
                    out=v[:], in0=tmp[:], scalar1=MAGIC, scalar2=MAGIC,
                    op0=add, op1=sub)
                v_prev = v

            if v_prev is not None:
                nc.sync.dma_start(out=d_out[:], in_=v_prev[:])

    nc.compile()
    return nc


def _pack_w(Wneg):
    """[N, N] -> [128, 16*N]: column block kc holds rows 128*kc.. of Wneg"""
    return np.ascontiguousarray(
        Wneg.reshape(16, 128, N).transpose(1, 0, 2).reshape(128, 16 * N))


def _host_constants(drive, r, eps, beta, K_local, W_cc, kernel_size, mode):
    """All scale folding in fp64, rounded to fp32 once."""
    pad = kernel_size // 2
    W64 = W_cc.astype(np.float64)
    C64 = np.zeros((N, N))
    idx = np.arange(N)
    for j in range(kernel_size):
        C64[(idx + j - pad) % N, idx] += np.float64(K_local[j])
    eps64 = eps.astype(np.float64)
    beta64 = beta.astype(np.float64)
    W3 = 0.5 * (1 - beta64)[None, :] * eps64[None, :] * (W64 + C64)
    W3[idx, idx] += (1 - beta64) * (1 - eps64)
    Wneg = (-127.0 * W3).astype(np.float32)          # mneg @ Wneg == mapped @ (127*W3)
    if mode != "fp16x3":
        wmaps = dict(Wp=_pack_w(Wneg))
    else:
        Wh = Wneg.astype(np.float16)
        Wl = ((Wneg - Wh.astype(np.float32)) * np.float32(LOSCALE)).astype(np.float16)
        wmaps = dict(Wh=_pack_w(Wh), Wl=_pack_w(Wl))
    # mneg = (g-1)*(r*g) = -mapped; the minus sign is folded into Wneg
    R1 = r.astype(np.float32)                                  # g-space step 0
    R2 = (r.astype(np.float64) / (127.0 * 127.0)).astype(np.float32)   # v-space
    return wmaps, R1, R2, beta64


def _to_bn(x):
    """[32, 2048] -> scrambled [128, 512]: bn[32*j + b, nt] = x[b, 512*j + nt]"""
    return np.ascontiguousarray(
        x.reshape(BL, 4, 512).transpose(1, 0, 2).reshape(128, 512))


def _from_bn(x):
    return np.ascontiguousarray(
        x.reshape(4, BL, 512).transpose(1, 0, 2).reshape(BL, N))


def _bcast_bn(site):
    """[2048] per-site constant -> scrambled [128, 512] (same for all b)"""
    return np.ascontiguousarray(np.broadcast_to(
        site.reshape(4, 1, 512), (4, BL, 512)).reshape(128, 512))


def kernel(drive, r, eps, beta, K_local, W_cc, steps=64, kernel_size=5, **_kw):
    from concourse.bass_utils import run_bass_kernel_spmd

    drive = np.asarray(drive, dtype=np.float32)
    r = np.asarray(r, dtype=np.float32)
    eps = np.asarray(eps, dtype=np.float32)
    beta = np.asarray(beta, dtype=np.float32)
    K_local = np.asarray(K_local, dtype=np.float32)
    W_cc = np.asarray(W_cc, dtype=np.float32)
    steps = int(steps)
    kernel_size = int(kernel_size)

    lo, hi = np.float32(0.0001), np.float32(1.0 - 0.0001)
    if steps <= 0:
        return np.clip(drive, lo, hi).astype(np.float32)

    wmaps, R1, R2, beta64 = _host_constants(
        drive, r, eps, beta, K_local, W_cc, kernel_size, MODE)
    R1_bn = _bcast_bn(R1)
    R2_bn = _bcast_bn(R2)
    ident = np.eye(128, dtype=np.float32)

    key = (steps, MODE)
    if key not in _programs:
        _programs[key] = _build_program(steps, mode=MODE)
    nc = _programs[key]

    in_maps = []
    for c in range(NCORES):
        dslice = drive[BL * c:BL * (c + 1)]
        C127 = (127.0 * beta64[None, :] * dslice.astype(np.float64)).astype(np.float32)
        in_maps.append(dict(
            g0_bn=_to_bn(dslice), C_bn=_to_bn(C127),
            R1_bn=R1_bn, R2_bn=R2_bn, ident=ident, **wmaps))

    global _last_in_maps
    _last_in_maps = in_maps
    res = run_bass_kernel_spmd(nc, in_maps, list(range(NCORES)))

    out = np.empty((B, N), dtype=np.float32)
    for c in range(NCORES):
        v = _from_bn(res.results[c]["v_out"])
        g = (v / np.float32(127.0)).astype(np.float32)
        out[BL * c:BL * (c + 1)] = np.clip(g, lo, hi)
    return out



# revision 5
# speedup vs baseline: 2.5132x; 2.5132x over previous
"""CML int8-sim Trainium2 kernel.

Strategy (data-parallel over batch, 8 cores, B=256 -> 32 rows/core):
The per-step update
    mapped = r*g*(1-g)
    coupled = 0.5*(circ_conv(mapped, K) + mapped @ W_cc)
    g' = (1-beta)*((1-eps)*mapped + eps*coupled) + beta*drive
    g  = round(127*g')/127
is linear in `mapped` after the logistic map, so the circular conv, the
global coupling, the eps/beta site scalings and the 127 quantization
scale all fold into one precomputed matrix W3:
    127*g' = mapped @ W3_127 + 127*beta*drive

Default mode "f32rd": the matmul runs in fp32r (hardware TF32-like
format: 8-bit exponent, 11-bit mantissa, 1 PE cycle/row vs fp32's 4),
but the *diagonal* of W3 (the large (1-beta)(1-eps) passthrough, ~0.73
vs ~5e-4 off-diagonal entries) is pulled out of the matmul and applied
exactly in fp32 on the vector engine, so quantization-boundary flips
stay near fp32 levels.  The logistic map is refactored around the ACT
engine's (exact) square function:
    mapped = (r/127^2) * (63.5^2 - (v-63.5)^2),  v = 127*g integer
which also feeds the diagonal/drive term tc = D.*sq + C2 computed on
the otherwise-idle GPSIMD engine.  Elementwise state is processed in
four 128-column quarters so each quarter's round->map->transpose tail
overlaps the PE matmul stream of other quarters/steps (matmuls are
issued block-major in the contraction index so the first quarter's
transposed block unblocks 4 of 16 rounds immediately).

State is kept 127-scaled ("v = 127*g", integer-valued fp32), in a
scrambled [128, 512] layout (partition = 32*ntile + batch) so that all
elementwise work uses the full 128 partitions.

Mode "f16d" is identical but stores W / streams matmuls in fp16.
Mode "fp32p" is the previous full-fp32 pipelined kernel (4x slower PE).
"""
import numpy as np

B, N, NCORES = 256, 2048, 8
BL = B // NCORES          # 32 batch rows per core
MAGIC = 12582912.0        # 1.5*2^23: (x+M)-M == RNE round for |x| < 2^22

MODE = "f16d"             # "f32rd" | "f16d" | "fp32p"
_programs = {}
_last_in_maps = None


def round_fp32r(a):
    """RNE-round fp32 array to fp32r (8-bit exp, 11-bit mantissa)."""
    u = np.asarray(a, dtype=np.float32).view(np.uint32).astype(np.uint64)
    u = u + 0x7FF + ((u >> 12) & 1)
    return (u & 0xFFFFF000).astype(np.uint32).view(np.float32)


def _build_program(steps, mode="f32rd"):
    import concourse.mybir as mybir
    import concourse.tile as tile
    from concourse import bacc

    f32 = mybir.dt.float32
    f32r = mybir.dt.float32r
    f16 = mybir.dt.float16
    sub = mybir.AluOpType.subtract
    add = mybir.AluOpType.add
    mult = mybir.AluOpType.mult

    nc = bacc.Bacc("TRN2", target_bir_lowering=False, debug=False)

    if mode == "fp32p":
        return _build_fp32p(nc, steps, mybir, tile)

    wdt = f32r if mode == "f32rd" else f16
    d_Wp = nc.dram_tensor("Wp", [128, 16 * N], wdt, kind="ExternalInput").ap()
    d_v0 = nc.dram_tensor("v0_bn", [128, 512], f32, kind="ExternalInput").ap()
    d_Q = nc.dram_tensor("Q_bn", [128, 512], f32, kind="ExternalInput").ap()
    d_D = nc.dram_tensor("D_bn", [128, 512], f32, kind="ExternalInput").ap()
    d_C2 = nc.dram_tensor("C2_bn", [128, 512], f32, kind="ExternalInput").ap()
    d_id = nc.dram_tensor("ident", [128, 128], f32, kind="ExternalInput").ap()
    d_out = nc.dram_tensor("v_out", [128, 512], f32, kind="ExternalOutput").ap()

    SQ_C = 4032.25  # 63.5^2

    with tile.TileContext(nc) as tc:
        with tc.tile_pool(name="consts", bufs=1) as cp, \
             tc.tile_pool(name="work", bufs=2) as wp, \
             tc.tile_pool(name="psum", bufs=2, space="PSUM") as pp:
            s_Wp = cp.tile([128, 16 * N], wdt)
            s_v0 = cp.tile([128, 512], f32)
            s_Q = cp.tile([128, 512], f32)
            s_D = cp.tile([128, 512], f32)
            s_C2 = cp.tile([128, 512], f32)
            s_id = cp.tile([128, 128], f32)
            s_b = cp.tile([128, 1], f32)
            nc.gpsimd.memset(s_b[:], -63.5)
            nc.sync.dma_start(out=s_v0[:], in_=d_v0[:])
            nc.sync.dma_start(out=s_Q[:], in_=d_Q[:])
            nc.sync.dma_start(out=s_D[:], in_=d_D[:])
            nc.sync.dma_start(out=s_C2[:], in_=d_C2[:])
            nc.sync.dma_start(out=s_id[:], in_=d_id[:])
            # W3 is big: DMA per 128-chunk so first matmuls can start
            # before the whole matrix has landed.
            for kc in range(16):
                nc.sync.dma_start(out=s_Wp[:, N * kc:N * (kc + 1)],
                                  in_=d_Wp[:, N * kc:N * (kc + 1)])

            # contraction-chunk order: block-major so the 4 rounds that
            # need only transposed quarter q run consecutively.
            ORDER = [0, 4, 8, 12, 1, 5, 9, 13, 2, 6, 10, 14, 3, 7, 11, 15]
            Sq = mybir.ActivationFunctionType.Square

            P1_prev = None
            tc_prev = None
            for t in range(steps):
                mts = []
                tcs = []
                pT = pp.tile([128, 512], f32, tag="pT")
                for q in range(4):
                    c0 = 128 * q
                    if t == 0:
                        v = s_v0[:, c0:c0 + 128]
                    else:
                        z = wp.tile([128, 128], f32, tag=f"z{q}")
                        nc.vector.tensor_add(
                            out=z[:], in0=P1_prev[:, c0:c0 + 128],
                            in1=tc_prev[q][:])
                        vq = wp.tile([128, 128], f32, tag=f"v{q}")
                        nc.vector.tensor_scalar(
                            out=vq[:], in0=z[:], scalar1=MAGIC, scalar2=MAGIC,
                            op0=add, op1=sub)
                        v = vq[:]
                    sq = wp.tile([128, 128], f32, tag=f"sq{q}")
                    nc.scalar.activation(sq[:], v, Sq, bias=s_b[:])
                    # tc for the *next* step's z: D.*sq + C2  (GPSIMD)
                    g1 = wp.tile([128, 128], f32, tag=f"g1{q}")
                    nc.gpsimd.tensor_tensor(
                        out=g1[:], in0=sq[:], in1=s_D[:, c0:c0 + 128], op=mult)
                    tcq = wp.tile([128, 128], f32, tag=f"tc{q}")
                    nc.gpsimd.tensor_tensor(
                        out=tcq[:], in0=g1[:], in1=s_C2[:, c0:c0 + 128], op=add)
                    tcs.append(tcq)
                    # mneg = (sq - 63.5^2) * (r/127^2)  == -mapped
                    mneg = wp.tile([128, 128], f32, tag=f"mneg{q}")
                    nc.vector.scalar_tensor_tensor(
                        out=mneg[:], in0=sq[:], scalar=SQ_C,
                        in1=s_Q[:, c0:c0 + 128], op0=sub, op1=mult)
                    nc.tensor.transpose(pT[:, c0:c0 + 128], mneg[:], s_id[:])
                    mt = wp.tile([128, 128], wdt, tag=f"mt{q}")
                    nc.scalar.copy(mt[:], pT[:, c0:c0 + 128])
                    mts.append(mt)

                P1 = pp.tile([128, 512], f32, tag="P1")
                for idx, kc in enumerate(ORDER):
                    lh = mts[kc % 4][:, 32 * (kc // 4):32 * (kc // 4) + 32]
                    for jj in range(4):
                        nc.tensor.matmul(
                            out=P1[32 * jj:32 * (jj + 1), :],
                            lhsT=lh, rhs=s_Wp[:, N * kc + 512 * jj:
                                              N * kc + 512 * (jj + 1)],
                            start=(idx == 0), stop=(idx == 15),
                            tile_position=(0, 32 * jj))
                P1_prev = P1
                tc_prev = tcs

            for q in range(4):
                c0 = 128 * q
                z = wp.tile([128, 128], f32, tag=f"z{q}")
                nc.vector.tensor_add(
                    out=z[:], in0=P1_prev[:, c0:c0 + 128], in1=tc_prev[q][:])
                vq = wp.tile([128, 128], f32, tag=f"v{q}")
                nc.vector.tensor_scalar(
                    out=vq[:], in0=z[:], scalar1=MAGIC, scalar2=MAGIC,
                    op0=add, op1=sub)
                nc.sync.dma_start(out=d_out[:, c0:c0 + 128], in_=vq[:])

    nc.compile()
    return nc


def _build_fp32p(nc, steps, mybir, tile):
    """Previous-generation full-fp32 pipelined kernel (fallback)."""
    f32 = mybir.dt.float32
    sub = mybir.AluOpType.subtract
    add = mybir.AluOpType.add
    mult = mybir.AluOpType.mult

    d_Wp = nc.dram_tensor("Wp", [128, 16 * N], f32, kind="ExternalInput").ap()
    d_g0 = nc.dram_tensor("g0_bn", [128, 512], f32, kind="ExternalInput").ap()
    d_C = nc.dram_tensor("C_bn", [128, 512], f32, kind="ExternalInput").ap()
    d_R1 = nc.dram_tensor("R1_bn", [128, 512], f32, kind="ExternalInput").ap()
    d_R2 = nc.dram_tensor("R2_bn", [128, 512], f32, kind="ExternalInput").ap()
    d_id = nc.dram_tensor("ident", [128, 128], f32, kind="ExternalInput").ap()
    d_out = nc.dram_tensor("v_out", [128, 512], f32, kind="ExternalOutput").ap()

    with tile.TileContext(nc) as tc:
        with tc.tile_pool(name="consts", bufs=1) as cp, \
             tc.tile_pool(name="work", bufs=2) as wp, \
             tc.tile_pool(name="psum", bufs=2, space="PSUM") as pp:
            s_Wp = cp.tile([128, 16 * N], f32)
            s_g0 = cp.tile([128, 512], f32)
            s_C = cp.tile([128, 512], f32)
            s_R1 = cp.tile([128, 512], f32)
            s_R2 = cp.tile([128, 512], f32)
            s_id = cp.tile([128, 128], f32)
            nc.sync.dma_start(out=s_g0[:], in_=d_g0[:])
            nc.sync.dma_start(out=s_C[:], in_=d_C[:])
            nc.sync.dma_start(out=s_R1[:], in_=d_R1[:])
            nc.sync.dma_start(out=s_R2[:], in_=d_R2[:])
            nc.sync.dma_start(out=s_id[:], in_=d_id[:])
            for kc in range(16):
                nc.sync.dma_start(out=s_Wp[:, N * kc:N * (kc + 1)],
                                  in_=d_Wp[:, N * kc:N * (kc + 1)])

            ORDER = [0, 4, 8, 12, 1, 5, 9, 13, 2, 6, 10, 14, 3, 7, 11, 15]
            vh_prev = None
            for t in range(steps):
                mts = []
                for h in (0, 1):
                    if t == 0:
                        src = s_g0[:, 256 * h:256 * (h + 1)]
                        Rt, shift = s_R1, 1.0
                    else:
                        src = vh_prev[h][:]
                        Rt, shift = s_R2, 127.0
                    a = wp.tile([128, 256], f32, tag=f"a{h}")
                    nc.vector.tensor_mul(
                        out=a[:], in0=Rt[:, 256 * h:256 * (h + 1)], in1=src)
                    mneg = wp.tile([128, 256], f32, tag=f"mneg{h}")
                    nc.vector.scalar_tensor_tensor(
                        out=mneg[:], in0=src, scalar=shift, in1=a[:],
                        op0=sub, op1=mult)
                    pT = pp.tile([128, 256], f32, tag=f"pT{h}")
                    for b in range(2):
                        nc.tensor.transpose(
                            pT[:, 128 * b:128 * (b + 1)],
                            mneg[:, 128 * b:128 * (b + 1)], s_id[:])
                    mt = wp.tile([128, 256], f32, tag=f"mTs{h}")
                    nc.scalar.copy(mt[:], pT[:])
                    mts.append(mt)

                def lhs(kc):
                    bk = kc % 4
                    off = 128 * (bk % 2) + 32 * (kc // 4)
                    return mts[bk // 2][:, off:off + 32]

                vh = []
                for h in (0, 1):
                    P1 = pp.tile([128, 256], f32, tag=f"P1h{h}")
                    for idx, kc in enumerate(ORDER):
                        lh = lhs(kc)
                        for j in range(4):
                            base = N * kc + 512 * j + 256 * h
                            nc.tensor.matmul(
                                out=P1[32 * j:32 * (j + 1), :],
                                lhsT=lh, rhs=s_Wp[:, base:base + 256],
                                start=(idx == 0), stop=(idx == 15),
                                tile_position=(0, 32 * j))
                    tmp = wp.tile([128, 256], f32, tag=f"tmp{h}")
                    nc.vector.tensor_add(
                        out=tmp[:], in0=P1[:],
                        in1=s_C[:, 256 * h:256 * (h + 1)])
                    v = wp.tile([128, 256], f32, tag=f"v{h}")
                    nc.vector.tensor_scalar(
                        out=v[:], in0=tmp[:], scalar1=MAGIC, scalar2=MAGIC,
                        op0=add, op1=sub)
                    vh.append(v)
                vh_prev = vh

            nc.sync.dma_start(out=d_out[:, 0:256], in_=vh_prev[0][:])
            nc.sync.dma_start(out=d_out[:, 256:512], in_=vh_prev[1][:])

    nc.compile()
    return nc


def _pack_w(Wneg):
    """[N, N] -> [128, 16*N]: column block kc holds rows 128*kc.. of Wneg"""
    return np.ascontiguousarray(
        Wneg.reshape(16, 128, N).transpose(1, 0, 2).reshape(128, 16 * N))


def _host_constants(r, eps, beta, K_local, W_cc, kernel_size, mode):
    """All scale folding in fp64, rounded to fp32 once."""
    pad = kernel_size // 2
    W64 = W_cc.astype(np.float64)
    C64 = np.zeros((N, N))
    idx = np.arange(N)
    for j in range(kernel_size):
        C64[(idx + j - pad) % N, idx] += np.float64(K_local[j])
    eps64 = eps.astype(np.float64)
    beta64 = beta.astype(np.float64)
    r64 = r.astype(np.float64)
    W3 = 0.5 * (1 - beta64)[None, :] * eps64[None, :] * (W64 + C64)
    W3[idx, idx] += (1 - beta64) * (1 - eps64)
    if mode == "fp32p":
        Wneg = (-127.0 * W3).astype(np.float32)
        R1 = r.astype(np.float32)
        R2 = (r64 / (127.0 * 127.0)).astype(np.float32)
        return dict(Wp=_pack_w(Wneg)), dict(R1=R1, R2=R2), beta64
    # diag-separated modes
    Ddiag = 127.0 * np.diag(W3).copy()         # [N]
    W3off = W3.copy()
    W3off[idx, idx] = 0.0
    Wneg = (-127.0 * W3off).astype(np.float32)  # mnegT.T @ Wneg == mapped@127*W3off
    if mode == "f32rd":
        wmaps = dict(Wp=_pack_w(round_fp32r(Wneg)))
    else:
        wmaps = dict(Wp=_pack_w(Wneg.astype(np.float16)))
    Q = (r64 / (127.0 * 127.0)).astype(np.float32)          # mneg scale
    Dneg64 = -Ddiag * r64 / (127.0 * 127.0)                 # tc = D.*sq + C2
    D = Dneg64.astype(np.float32)
    C2site64 = -4032.25 * Dneg64                            # + C (per-batch) later
    return wmaps, dict(Q=Q, D=D, C2site=C2site64), beta64


def _to_bn(x):
    """[32, 2048] -> scrambled [128, 512]: bn[32*j + b, nt] = x[b, 512*j + nt]"""
    return np.ascontiguousarray(
        x.reshape(BL, 4, 512).transpose(1, 0, 2).reshape(128, 512))


def _from_bn(x):
    return np.ascontiguousarray(
        x.reshape(4, BL, 512).transpose(1, 0, 2).reshape(BL, N))


def _bcast_bn(site):
    """[2048] per-site constant -> scrambled [128, 512] (same for all b)"""
    return np.ascontiguousarray(np.broadcast_to(
        site.reshape(4, 1, 512), (4, BL, 512)).reshape(128, 512))


def kernel(drive, r, eps, beta, K_local, W_cc, steps=64, kernel_size=5, **_kw):
    from concourse.bass_utils import run_bass_kernel_spmd

    drive = np.asarray(drive, dtype=np.float32)
    r = np.asarray(r, dtype=np.float32)
    eps = np.asarray(eps, dtype=np.float32)
    beta = np.asarray(beta, dtype=np.float32)
    K_local = np.asarray(K_local, dtype=np.float32)
    W_cc = np.asarray(W_cc, dtype=np.float32)
    steps = int(steps)
    kernel_size = int(kernel_size)

    lo, hi = np.float32(0.0001), np.float32(1.0 - 0.0001)
    if steps <= 0:
        return np.clip(drive, lo, hi).astype(np.float32)

    wmaps, consts, beta64 = _host_constants(
        r, eps, beta, K_local, W_cc, kernel_size, MODE)
    ident = np.eye(128, dtype=np.float32)

    key = (steps, MODE)
    if key not in _programs:
        _programs[key] = _build_program(steps, mode=MODE)
    nc = _programs[key]

    in_maps = []
    for c in range(NCORES):
        dslice = drive[BL * c:BL * (c + 1)]
        C127 = 127.0 * beta64[None, :] * dslice.astype(np.float64)
        if MODE == "fp32p":
            in_maps.append(dict(
                g0_bn=_to_bn(dslice), C_bn=_to_bn(C127.astype(np.float32)),
                R1_bn=_bcast_bn(consts["R1"]), R2_bn=_bcast_bn(consts["R2"]),
                ident=ident, **wmaps))
        else:
            v0 = (127.0 * dslice).astype(np.float32)
            C2 = (C127 + consts["C2site"][None, :]).astype(np.float32)
            in_maps.append(dict(
                v0_bn=_to_bn(v0), C2_bn=_to_bn(C2),
                Q_bn=_bcast_bn(consts["Q"]), D_bn=_bcast_bn(consts["D"]),
                ident=ident, **wmaps))

    global _last_in_maps
    _last_in_maps = in_maps
    res = run_bass_kernel_spmd(nc, in_maps, list(range(NCORES)))

    out = np.empty((B, N), dtype=np.float32)
    for c in range(NCORES):
        v = _from_bn(res.results[c]["v_out"])
        g = (v / np.float32(127.0)).astype(np.float32)
        out[BL * c:BL * (c + 1)] = np.clip(g, lo, hi)
    return out


# revision 8
# speedup vs baseline: 2.5654x; 1.0208x over previous
"""CML int8-sim Trainium2 kernel.

Strategy (data-parallel over batch, 8 cores, B=256 -> 32 rows/core):
The per-step update
    mapped = r*g*(1-g)
    coupled = 0.5*(circ_conv(mapped, K) + mapped @ W_cc)
    g' = (1-beta)*((1-eps)*mapped + eps*coupled) + beta*drive
    g  = round(127*g')/127
is linear in `mapped` after the logistic map, so the circular conv, the
global coupling, the eps/beta site scalings and the 127 quantization
scale all fold into one precomputed matrix W3:
    127*g' = mapped @ W3_127 + 127*beta*drive

Default mode "f32rd": the matmul runs in fp32r (hardware TF32-like
format: 8-bit exponent, 11-bit mantissa, 1 PE cycle/row vs fp32's 4),
but the *diagonal* of W3 (the large (1-beta)(1-eps) passthrough, ~0.73
vs ~5e-4 off-diagonal entries) is pulled out of the matmul and applied
exactly in fp32 on the vector engine, so quantization-boundary flips
stay near fp32 levels.  The logistic map is refactored around the ACT
engine's (exact) square function:
    mapped = (r/127^2) * (63.5^2 - (v-63.5)^2),  v = 127*g integer
which also feeds the diagonal/drive term tc = D.*sq + C2 computed on
the otherwise-idle GPSIMD engine.  Elementwise state is processed in
four 128-column quarters so each quarter's round->map->transpose tail
overlaps the PE matmul stream of other quarters/steps (matmuls are
issued block-major in the contraction index so the first quarter's
transposed block unblocks 4 of 16 rounds immediately).

State is kept 127-scaled ("v = 127*g", integer-valued fp32), in a
scrambled [128, 512] layout (partition = 32*ntile + batch) so that all
elementwise work uses the full 128 partitions.

Mode "f16d" is identical but stores W / streams matmuls in fp16.
Mode "fp32p" is the previous full-fp32 pipelined kernel (4x slower PE).
"""
import numpy as np

B, N, NCORES = 256, 2048, 8
BL = B // NCORES          # 32 batch rows per core
MAGIC = 12582912.0        # 1.5*2^23: (x+M)-M == RNE round for |x| < 2^22

MODE = "f16d"             # "f32rd" | "f16d" | "fp32p"
_programs = {}
_last_in_maps = None


def round_fp32r(a):
    """RNE-round fp32 array to fp32r (8-bit exp, 11-bit mantissa)."""
    u = np.asarray(a, dtype=np.float32).view(np.uint32).astype(np.uint64)
    u = u + 0x7FF + ((u >> 12) & 1)
    return (u & 0xFFFFF000).astype(np.uint32).view(np.float32)


def _build_program(steps, mode="f32rd"):
    import concourse.mybir as mybir
    import concourse.tile as tile
    from concourse import bacc

    f32 = mybir.dt.float32
    f32r = mybir.dt.float32r
    f16 = mybir.dt.float16
    sub = mybir.AluOpType.subtract
    add = mybir.AluOpType.add
    mult = mybir.AluOpType.mult

    nc = bacc.Bacc("TRN2", target_bir_lowering=False, debug=False)

    if mode == "fp32p":
        return _build_fp32p(nc, steps, mybir, tile)

    d_Wp = nc.dram_tensor("Wp", [128, 16 * N], f16, kind="ExternalInput").ap()
    d_v0 = nc.dram_tensor("v0_bn", [128, 512], f32, kind="ExternalInput").ap()
    d_D = nc.dram_tensor("D_bn", [128, 512], f32, kind="ExternalInput").ap()
    d_C2 = nc.dram_tensor("C2_bn", [128, 512], f32, kind="ExternalInput").ap()
    d_id = nc.dram_tensor("ident", [128, 128], f32, kind="ExternalInput").ap()
    d_out = nc.dram_tensor("v_out", [128, 512], f32, kind="ExternalOutput").ap()

    with tile.TileContext(nc) as tc:
        with tc.tile_pool(name="consts", bufs=1) as cp, \
             tc.tile_pool(name="work", bufs=2) as wp, \
             tc.tile_pool(name="psum", bufs=2, space="PSUM") as pp:
            s_Wp = cp.tile([128, 16 * N], f16)
            s_v0 = cp.tile([128, 512], f32)
            s_D = cp.tile([128, 512], f32)
            s_C2 = cp.tile([128, 512], f32)
            s_id = cp.tile([128, 128], f32)
            s_b = cp.tile([128, 1], f32)
            nc.gpsimd.memset(s_b[:], -63.5)
            nc.sync.dma_start(out=s_v0[:], in_=d_v0[:])
            nc.sync.dma_start(out=s_D[:], in_=d_D[:])
            nc.sync.dma_start(out=s_C2[:], in_=d_C2[:])
            nc.sync.dma_start(out=s_id[:], in_=d_id[:])
            # W2 is big: DMA per 128-chunk so first matmuls can start
            # before the whole matrix has landed.
            for kc in range(16):
                nc.sync.dma_start(out=s_Wp[:, N * kc:N * (kc + 1)],
                                  in_=d_Wp[:, N * kc:N * (kc + 1)])

            # contraction-chunk order: block-major so the 4 rounds that
            # need only transposed quarter q run consecutively.
            ORDER = [0, 4, 8, 12, 1, 5, 9, 13, 2, 6, 10, 14, 3, 7, 11, 15]
            Sq = mybir.ActivationFunctionType.Square

            def tail_q(q, P1h_prev, tc_prev):
                """z -> v for quarter q from previous-step P1 half + tc."""
                hq, cq = q // 2, 128 * (q % 2)
                z = wp.tile([128, 128], f32, tag=f"z{q}", name="z")
                nc.vector.tensor_add(
                    out=z[:], in0=P1h_prev[hq][:, cq:cq + 128],
                    in1=tc_prev[q][:])
                vq = wp.tile([128, 128], f32, tag=f"v{q}", name="vq")
                nc.vector.tensor_scalar(
                    out=vq[:], in0=z[:], scalar1=MAGIC, scalar2=MAGIC,
                    op0=add, op1=sub)
                return vq

            P1h_prev = None
            tc_prev = None
            for t in range(steps):
                mts = []
                tcs = []
                pT = pp.tile([128, 512], f32, tag="pT", name="pT")
                for q in range(4):
                    c0 = 128 * q
                    if t == 0:
                        v = s_v0[:, c0:c0 + 128]
                    else:
                        v = tail_q(q, P1h_prev, tc_prev)[:]
                    sq = wp.tile([128, 128], f32, tag=f"sq{q}")
                    nc.scalar.activation(sq[:], v, Sq, bias=s_b[:])
                    # tc for the *next* step's z: D.*sq + C2  (GPSIMD)
                    g1 = wp.tile([128, 128], f32, tag=f"g1{q}")
                    nc.gpsimd.tensor_tensor(
                        out=g1[:], in0=sq[:], in1=s_D[:, c0:c0 + 128], op=mult)
                    tcq = wp.tile([128, 128], f32, tag=f"tc{q}")
                    nc.gpsimd.tensor_tensor(
                        out=tcq[:], in0=g1[:], in1=s_C2[:, c0:c0 + 128], op=add)
                    tcs.append(tcq)
                    # lhsT operand = transpose(sq)/256 in fp16; the 1/256
                    # rescale (folded into W2 host-side) keeps fp16 W
                    # entries out of the subnormal range.
                    nc.tensor.transpose(pT[:, c0:c0 + 128], sq[:], s_id[:])
                    mt = wp.tile([128, 128], f16, tag=f"mt{q}")
                    nc.scalar.mul(mt[:], pT[:, c0:c0 + 128], 1.0 / 256.0)
                    mts.append(mt)

                P1h = []
                for h in (0, 1):
                    P1 = pp.tile([128, 256], f32, tag=f"P1h{h}", name="P1")
                    for idx, kc in enumerate(ORDER):
                        lh = mts[kc % 4][:, 32 * (kc // 4):32 * (kc // 4) + 32]
                        for jj in range(4):
                            base = N * kc + 512 * jj + 256 * h
                            nc.tensor.matmul(
                                out=P1[32 * jj:32 * (jj + 1), :],
                                lhsT=lh, rhs=s_Wp[:, base:base + 256],
                                start=(idx == 0), stop=(idx == 15),
                                tile_position=(0, 32 * jj))
                    P1h.append(P1)
                P1h_prev = P1h
                tc_prev = tcs

            for q in range(4):
                vq = tail_q(q, P1h_prev, tc_prev)
                nc.sync.dma_start(out=d_out[:, 128 * q:128 * q + 128],
                                  in_=vq[:])

    nc.compile()
    return nc


def _build_fp32p(nc, steps, mybir, tile):
    """Previous-generation full-fp32 pipelined kernel (fallback)."""
    f32 = mybir.dt.float32
    sub = mybir.AluOpType.subtract
    add = mybir.AluOpType.add
    mult = mybir.AluOpType.mult

    d_Wp = nc.dram_tensor("Wp", [128, 16 * N], f32, kind="ExternalInput").ap()
    d_g0 = nc.dram_tensor("g0_bn", [128, 512], f32, kind="ExternalInput").ap()
    d_C = nc.dram_tensor("C_bn", [128, 512], f32, kind="ExternalInput").ap()
    d_R1 = nc.dram_tensor("R1_bn", [128, 512], f32, kind="ExternalInput").ap()
    d_R2 = nc.dram_tensor("R2_bn", [128, 512], f32, kind="ExternalInput").ap()
    d_id = nc.dram_tensor("ident", [128, 128], f32, kind="ExternalInput").ap()
    d_out = nc.dram_tensor("v_out", [128, 512], f32, kind="ExternalOutput").ap()

    with tile.TileContext(nc) as tc:
        with tc.tile_pool(name="consts", bufs=1) as cp, \
             tc.tile_pool(name="work", bufs=2) as wp, \
             tc.tile_pool(name="psum", bufs=2, space="PSUM") as pp:
            s_Wp = cp.tile([128, 16 * N], f32)
            s_g0 = cp.tile([128, 512], f32)
            s_C = cp.tile([128, 512], f32)
            s_R1 = cp.tile([128, 512], f32)
            s_R2 = cp.tile([128, 512], f32)
            s_id = cp.tile([128, 128], f32)
            nc.sync.dma_start(out=s_g0[:], in_=d_g0[:])
            nc.sync.dma_start(out=s_C[:], in_=d_C[:])
            nc.sync.dma_start(out=s_R1[:], in_=d_R1[:])
            nc.sync.dma_start(out=s_R2[:], in_=d_R2[:])
            nc.sync.dma_start(out=s_id[:], in_=d_id[:])
            for kc in range(16):
                nc.sync.dma_start(out=s_Wp[:, N * kc:N * (kc + 1)],
                                  in_=d_Wp[:, N * kc:N * (kc + 1)])

            ORDER = [0, 4, 8, 12, 1, 5, 9, 13, 2, 6, 10, 14, 3, 7, 11, 15]
            vh_prev = None
            for t in range(steps):
                mts = []
                for h in (0, 1):
                    if t == 0:
                        src = s_g0[:, 256 * h:256 * (h + 1)]
                        Rt, shift = s_R1, 1.0
                    else:
                        src = vh_prev[h][:]
                        Rt, shift = s_R2, 127.0
                    a = wp.tile([128, 256], f32, tag=f"a{h}")
                    nc.vector.tensor_mul(
                        out=a[:], in0=Rt[:, 256 * h:256 * (h + 1)], in1=src)
                    mneg = wp.tile([128, 256], f32, tag=f"mneg{h}")
                    nc.vector.scalar_tensor_tensor(
                        out=mneg[:], in0=src, scalar=shift, in1=a[:],
                        op0=sub, op1=mult)
                    pT = pp.tile([128, 256], f32, tag=f"pT{h}")
                    for b in range(2):
                        nc.tensor.transpose(
                            pT[:, 128 * b:128 * (b + 1)],
                            mneg[:, 128 * b:128 * (b + 1)], s_id[:])
                    mt = wp.tile([128, 256], f32, tag=f"mTs{h}")
                    nc.scalar.copy(mt[:], pT[:])
                    mts.append(mt)

                def lhs(kc):
                    bk = kc % 4
                    off = 128 * (bk % 2) + 32 * (kc // 4)
                    return mts[bk // 2][:, off:off + 32]

                vh = []
                for h in (0, 1):
                    P1 = pp.tile([128, 256], f32, tag=f"P1h{h}")
                    for idx, kc in enumerate(ORDER):
                        lh = lhs(kc)
                        for j in range(4):
                            base = N * kc + 512 * j + 256 * h
                            nc.tensor.matmul(
                                out=P1[32 * j:32 * (j + 1), :],
                                lhsT=lh, rhs=s_Wp[:, base:base + 256],
                                start=(idx == 0), stop=(idx == 15),
                                tile_position=(0, 32 * j))
                    tmp = wp.tile([128, 256], f32, tag=f"tmp{h}")
                    nc.vector.tensor_add(
                        out=tmp[:], in0=P1[:],
                        in1=s_C[:, 256 * h:256 * (h + 1)])
                    v = wp.tile([128, 256], f32, tag=f"v{h}")
                    nc.vector.tensor_scalar(
                        out=v[:], in0=tmp[:], scalar1=MAGIC, scalar2=MAGIC,
                        op0=add, op1=sub)
                    vh.append(v)
                vh_prev = vh

            nc.sync.dma_start(out=d_out[:, 0:256], in_=vh_prev[0][:])
            nc.sync.dma_start(out=d_out[:, 256:512], in_=vh_prev[1][:])

    nc.compile()
    return nc


def _pack_w(Wneg):
    """[N, N] -> [128, 16*N]: column block kc holds rows 128*kc.. of Wneg"""
    return np.ascontiguousarray(
        Wneg.reshape(16, 128, N).transpose(1, 0, 2).reshape(128, 16 * N))


def _host_constants(r, eps, beta, K_local, W_cc, kernel_size, mode):
    """All scale folding in fp64, rounded to fp32 once."""
    pad = kernel_size // 2
    W64 = W_cc.astype(np.float64)
    C64 = np.zeros((N, N))
    idx = np.arange(N)
    for j in range(kernel_size):
        C64[(idx + j - pad) % N, idx] += np.float64(K_local[j])
    eps64 = eps.astype(np.float64)
    beta64 = beta.astype(np.float64)
    r64 = r.astype(np.float64)
    W3 = 0.5 * (1 - beta64)[None, :] * eps64[None, :] * (W64 + C64)
    W3[idx, idx] += (1 - beta64) * (1 - eps64)
    if mode == "fp32p":
        Wneg = (-127.0 * W3).astype(np.float32)
        R1 = r.astype(np.float32)
        R2 = (r64 / (127.0 * 127.0)).astype(np.float32)
        return dict(Wp=_pack_w(Wneg)), dict(R1=R1, R2=R2), beta64
    # diag-separated fp16 mode: W2[k,n] = -256*(r_k/127^2)*127*W3off[k,n]
    # so that P1 = (sq/256) @ W2 equals the off-diagonal part of
    # mapped @ (127*W3) minus the constant S (folded into C2).
    Ddiag = 127.0 * np.diag(W3).copy()         # [N]
    W3off = W3.copy()
    W3off[idx, idx] = 0.0
    Q64 = r64 / (127.0 * 127.0)
    W2 = -256.0 * 127.0 * Q64[:, None] * W3off
    wmaps = dict(Wp=_pack_w(W2.astype(np.float16)))
    S64 = 4032.25 * 127.0 * (Q64 @ W3off)                   # [N]
    Dneg64 = -Ddiag * Q64                                   # tc = D.*sq + C2
    D = Dneg64.astype(np.float32)
    C2site64 = S64 - 4032.25 * Dneg64                       # + C (per-batch) later
    return wmaps, dict(D=D, C2site=C2site64), beta64


def _to_bn(x):
    """[32, 2048] -> scrambled [128, 512]: bn[32*j + b, nt] = x[b, 512*j + nt]"""
    return np.ascontiguousarray(
        x.reshape(BL, 4, 512).transpose(1, 0, 2).reshape(128, 512))


def _from_bn(x):
    return np.ascontiguousarray(
        x.reshape(4, BL, 512).transpose(1, 0, 2).reshape(BL, N))


def _bcast_bn(site):
    """[2048] per-site constant -> scrambled [128, 512] (same for all b)"""
    return np.ascontiguousarray(np.broadcast_to(
        site.reshape(4, 1, 512), (4, BL, 512)).reshape(128, 512))


def kernel(drive, r, eps, beta, K_local, W_cc, steps=64, kernel_size=5, **_kw):
    from concourse.bass_utils import run_bass_kernel_spmd

    drive = np.asarray(drive, dtype=np.float32)
    r = np.asarray(r, dtype=np.float32)
    eps = np.asarray(eps, dtype=np.float32)
    beta = np.asarray(beta, dtype=np.float32)
    K_local = np.asarray(K_local, dtype=np.float32)
    W_cc = np.asarray(W_cc, dtype=np.float32)
    steps = int(steps)
    kernel_size = int(kernel_size)

    lo, hi = np.float32(0.0001), np.float32(1.0 - 0.0001)
    if steps <= 0:
        return np.clip(drive, lo, hi).astype(np.float32)

    wmaps, consts, beta64 = _host_constants(
        r, eps, beta, K_local, W_cc, kernel_size, MODE)
    ident = np.eye(128, dtype=np.float32)

    key = (steps, MODE)
    if key not in _programs:
        _programs[key] = _build_program(steps, mode=MODE)
    nc = _programs[key]

    in_maps = []
    for c in range(NCORES):
        dslice = drive[BL * c:BL * (c + 1)]
        C127 = 127.0 * beta64[None, :] * dslice.astype(np.float64)
        if MODE == "fp32p":
            in_maps.append(dict(
                g0_bn=_to_bn(dslice), C_bn=_to_bn(C127.astype(np.float32)),
                R1_bn=_bcast_bn(consts["R1"]), R2_bn=_bcast_bn(consts["R2"]),
                ident=ident, **wmaps))
        else:
            v0 = (127.0 * dslice).astype(np.float32)
            C2 = (C127 + consts["C2site"][None, :]).astype(np.float32)
            in_maps.append(dict(
                v0_bn=_to_bn(v0), C2_bn=_to_bn(C2),
                D_bn=_bcast_bn(consts["D"]),
                ident=ident, **wmaps))

    global _last_in_maps
    _last_in_maps = in_maps
    res = run_bass_kernel_spmd(nc, in_maps, list(range(NCORES)))

    out = np.empty((B, N), dtype=np.float32)
    for c in range(NCORES):
        v = _from_bn(res.results[c]["v_out"])
        g = (v / np.float32(127.0)).astype(np.float32)
        out[BL * c:BL * (c + 1)] = np.clip(g, lo, hi)
    return out


# revision 11
# speedup vs baseline: 3.8081x; 1.4844x over previous
"""CML int8-sim Trainium2 kernel.

Strategy (data-parallel over batch, 8 cores, B=256 -> 32 rows/core):
The per-step update
    mapped = r*g*(1-g)
    coupled = 0.5*(circ_conv(mapped, K) + mapped @ W_cc)
    g' = (1-beta)*((1-eps)*mapped + eps*coupled) + beta*drive
    g  = round(127*g')/127
is linear in `mapped` after the logistic map, so the circular conv, the
global coupling, the eps/beta site scalings and the 127 quantization
scale all fold into one precomputed matrix W3:
    127*g' = mapped @ W3_127 + 127*beta*drive

Default mode "f32rd": the matmul runs in fp32r (hardware TF32-like
format: 8-bit exponent, 11-bit mantissa, 1 PE cycle/row vs fp32's 4),
but the *diagonal* of W3 (the large (1-beta)(1-eps) passthrough, ~0.73
vs ~5e-4 off-diagonal entries) is pulled out of the matmul and applied
exactly in fp32 on the vector engine, so quantization-boundary flips
stay near fp32 levels.  The logistic map is refactored around the ACT
engine's (exact) square function:
    mapped = (r/127^2) * (63.5^2 - (v-63.5)^2),  v = 127*g integer
which also feeds the diagonal/drive term tc = D.*sq + C2 computed on
the otherwise-idle GPSIMD engine.  Elementwise state is processed in
four 128-column quarters so each quarter's round->map->transpose tail
overlaps the PE matmul stream of other quarters/steps (matmuls are
issued block-major in the contraction index so the first quarter's
transposed block unblocks 4 of 16 rounds immediately).

State is kept 127-scaled ("v = 127*g", integer-valued fp32), in a
scrambled [128, 512] layout (partition = 32*ntile + batch) so that all
elementwise work uses the full 128 partitions.

Mode "f16d" is identical but stores W / streams matmuls in fp16.
Mode "fp32p" is the previous full-fp32 pipelined kernel (4x slower PE).
"""
import numpy as np

B, N, NCORES = 256, 2048, 8
BL = B // NCORES          # 32 batch rows per core
MAGIC = 12582912.0        # 1.5*2^23: (x+M)-M == RNE round for |x| < 2^22

MODE = "f16d"             # "f32rd" | "f16d" | "fp32p"
_programs = {}
_last_in_maps = None


def round_fp32r(a):
    """RNE-round fp32 array to fp32r (8-bit exp, 11-bit mantissa)."""
    u = np.asarray(a, dtype=np.float32).view(np.uint32).astype(np.uint64)
    u = u + 0x7FF + ((u >> 12) & 1)
    return (u & 0xFFFFF000).astype(np.uint32).view(np.float32)


def _build_program(steps, mode="f32rd"):
    import concourse.mybir as mybir
    import concourse.tile as tile
    from concourse import bacc

    f32 = mybir.dt.float32
    f32r = mybir.dt.float32r
    f16 = mybir.dt.float16
    sub = mybir.AluOpType.subtract
    add = mybir.AluOpType.add
    mult = mybir.AluOpType.mult

    nc = bacc.Bacc("TRN2", target_bir_lowering=False, debug=False)

    if mode == "fp32p":
        return _build_fp32p(nc, steps, mybir, tile)

    d_Wp = nc.dram_tensor("Wp", [128, 16 * N], f16, kind="ExternalInput").ap()
    d_v0 = nc.dram_tensor("v0_bn", [128, 512], f32, kind="ExternalInput").ap()
    d_D = nc.dram_tensor("D_bn", [128, 512], f32, kind="ExternalInput").ap()
    d_C2 = nc.dram_tensor("C2_bn", [128, 512], f32, kind="ExternalInput").ap()
    d_id = nc.dram_tensor("ident", [128, 128], f32, kind="ExternalInput").ap()
    d_id16 = nc.dram_tensor("ident16", [128, 128], f16,
                            kind="ExternalInput").ap()
    d_out = nc.dram_tensor("v_out", [128, 512], f32, kind="ExternalOutput").ap()

    with tile.TileContext(nc) as tc:
        with tc.tile_pool(name="consts", bufs=1) as cp, \
             tc.tile_pool(name="work", bufs=2) as wp, \
             tc.tile_pool(name="psum", bufs=2, space="PSUM") as pp:
            s_Wp = cp.tile([128, 16 * N], f16)
            s_v0 = cp.tile([128, 512], f32)
            s_D = cp.tile([128, 512], f32)
            s_C2 = cp.tile([128, 512], f32)
            s_id = cp.tile([128, 128], f32)
            s_id16 = cp.tile([128, 128], f16)
            s_b = cp.tile([128, 1], f32)
            s_b2 = cp.tile([128, 1], f32)
            nc.gpsimd.memset(s_b[:], -63.5)
            nc.gpsimd.memset(s_b2[:], -3.96875)   # -63.5/16
            nc.sync.dma_start(out=s_v0[:], in_=d_v0[:])
            nc.sync.dma_start(out=s_D[:], in_=d_D[:])
            nc.sync.dma_start(out=s_C2[:], in_=d_C2[:])
            nc.sync.dma_start(out=s_id[:], in_=d_id[:])
            nc.sync.dma_start(out=s_id16[:], in_=d_id16[:])
            # W2 is big: DMA per 128-chunk so first matmuls can start
            # before the whole matrix has landed.
            for kc in range(16):
                nc.sync.dma_start(out=s_Wp[:, N * kc:N * (kc + 1)],
                                  in_=d_Wp[:, N * kc:N * (kc + 1)])

            # contraction-chunk order: block-major so the 4 rounds that
            # need only transposed quarter q run consecutively.
            ORDER = [0, 4, 8, 12, 1, 5, 9, 13, 2, 6, 10, 14, 3, 7, 11, 15]
            Sq = mybir.ActivationFunctionType.Square

            def tail_q(q, P1h_prev, tc_prev, out16):
                """z -> v for quarter q from previous-step P1 half + tc."""
                hq, cq = q // 2, 128 * (q % 2)
                z = wp.tile([128, 128], f32, tag=f"z{q}", name="z")
                nc.vector.tensor_add(
                    out=z[:], in0=P1h_prev[hq][:, cq:cq + 128],
                    in1=tc_prev[q][:])
                vq = wp.tile([128, 128], f16 if out16 else f32,
                             tag=f"v{q}", name="vq")
                nc.vector.tensor_scalar(
                    out=vq[:], in0=z[:], scalar1=MAGIC, scalar2=MAGIC,
                    op0=add, op1=sub)
                return vq

            s_v016 = cp.tile([128, 512], f16)
            nc.vector.tensor_copy(out=s_v016[:], in_=s_v0[:])

            P1h_prev = None
            tc_prev = None
            for t in range(steps):
                mts = [None] * 4
                tcs = [None] * 4
                pT = pp.tile([128, 512], f16, tag="pT", name="pT")

                def quarter(q, t=t, pT=pT, mts=mts, tcs=tcs,
                            P1h_prev=P1h_prev, tc_prev=tc_prev):
                    c0 = 128 * q
                    if t == 0:
                        v = s_v0[:, c0:c0 + 128]
                        v16 = s_v016[:, c0:c0 + 128]
                    else:
                        v16 = tail_q(q, P1h_prev, tc_prev, out16=True)[:]
                        v = v16
                    nc.tensor.transpose(pT[:, c0:c0 + 128], v16, s_id16[:])
                    # critical path: fp16 matmul operand = ((vT-63.5)/16)^2
                    mt = wp.tile([128, 128], f16, tag=f"mt{q}", name="mt")
                    nc.scalar.activation(mt[:], pT[:, c0:c0 + 128], Sq,
                                         bias=s_b2[:], scale=0.0625)
                    mts[q] = mt
                    # off-critical: exact sq for the diagonal/drive term of
                    # the *next* step's z (GPSIMD): tc = D.*sq + C2
                    sq = wp.tile([128, 128], f32, tag=f"sq{q}", name="sq")
                    nc.scalar.activation(sq[:], v, Sq, bias=s_b[:])
                    g1 = wp.tile([128, 128], f32, tag=f"g1{q}", name="g1")
                    nc.gpsimd.tensor_tensor(
                        out=g1[:], in0=sq[:], in1=s_D[:, c0:c0 + 128], op=mult)
                    tcq = wp.tile([128, 128], f32, tag=f"tc{q}", name="tcq")
                    nc.gpsimd.tensor_tensor(
                        out=tcq[:], in0=g1[:], in1=s_C2[:, c0:c0 + 128],
                        op=add)
                    tcs[q] = tcq

                def mm_rounds(h, P1, idxs):
                    for idx in idxs:
                        kc = ORDER[idx]
                        lh = mts[kc % 4][:, 32 * (kc // 4):32 * (kc // 4) + 32]
                        for jj in range(4):
                            base = N * kc + 512 * jj + 256 * h
                            nc.tensor.matmul(
                                out=P1[32 * jj:32 * (jj + 1), :],
                                lhsT=lh, rhs=s_Wp[:, base:base + 256],
                                start=(idx == 0), stop=(idx == 15),
                                tile_position=(0, 32 * jj))

                # Emission order = PE stream order: transposes for quarters
                # 0,1 ride right after the previous step's matmuls, the h0
                # rounds that need only those blocks run while quarters 2,3
                # (gated on the previous step's second half) catch up.
                quarter(0)
                quarter(1)
                P1a = pp.tile([128, 256], f32, tag="P1h0", name="P1a")
                mm_rounds(0, P1a, range(8))
                quarter(2)
                quarter(3)
                mm_rounds(0, P1a, range(8, 16))
                P1b = pp.tile([128, 256], f32, tag="P1h1", name="P1b")
                mm_rounds(1, P1b, range(16))
                P1h_prev = [P1a, P1b]
                tc_prev = tcs

            for q in range(4):
                vq = tail_q(q, P1h_prev, tc_prev, out16=False)
                nc.sync.dma_start(out=d_out[:, 128 * q:128 * q + 128],
                                  in_=vq[:])

    nc.compile()
    return nc


def _build_fp32p(nc, steps, mybir, tile):
    """Previous-generation full-fp32 pipelined kernel (fallback)."""
    f32 = mybir.dt.float32
    sub = mybir.AluOpType.subtract
    add = mybir.AluOpType.add
    mult = mybir.AluOpType.mult

    d_Wp = nc.dram_tensor("Wp", [128, 16 * N], f32, kind="ExternalInput").ap()
    d_g0 = nc.dram_tensor("g0_bn", [128, 512], f32, kind="ExternalInput").ap()
    d_C = nc.dram_tensor("C_bn", [128, 512], f32, kind="ExternalInput").ap()
    d_R1 = nc.dram_tensor("R1_bn", [128, 512], f32, kind="ExternalInput").ap()
    d_R2 = nc.dram_tensor("R2_bn", [128, 512], f32, kind="ExternalInput").ap()
    d_id = nc.dram_tensor("ident", [128, 128], f32, kind="ExternalInput").ap()
    d_out = nc.dram_tensor("v_out", [128, 512], f32, kind="ExternalOutput").ap()

    with tile.TileContext(nc) as tc:
        with tc.tile_pool(name="consts", bufs=1) as cp, \
             tc.tile_pool(name="work", bufs=2) as wp, \
             tc.tile_pool(name="psum", bufs=2, space="PSUM") as pp:
            s_Wp = cp.tile([128, 16 * N], f32)
            s_g0 = cp.tile([128, 512], f32)
            s_C = cp.tile([128, 512], f32)
            s_R1 = cp.tile([128, 512], f32)
            s_R2 = cp.tile([128, 512], f32)
            s_id = cp.tile([128, 128], f32)
            nc.sync.dma_start(out=s_g0[:], in_=d_g0[:])
            nc.sync.dma_start(out=s_C[:], in_=d_C[:])
            nc.sync.dma_start(out=s_R1[:], in_=d_R1[:])
            nc.sync.dma_start(out=s_R2[:], in_=d_R2[:])
            nc.sync.dma_start(out=s_id[:], in_=d_id[:])
            for kc in range(16):
                nc.sync.dma_start(out=s_Wp[:, N * kc:N * (kc + 1)],
                                  in_=d_Wp[:, N * kc:N * (kc + 1)])

            ORDER = [0, 4, 8, 12, 1, 5, 9, 13, 2, 6, 10, 14, 3, 7, 11, 15]
            vh_prev = None
            for t in range(steps):
                mts = []
                for h in (0, 1):
                    if t == 0:
                        src = s_g0[:, 256 * h:256 * (h + 1)]
                        Rt, shift = s_R1, 1.0
                    else:
                        src = vh_prev[h][:]
                        Rt, shift = s_R2, 127.0
                    a = wp.tile([128, 256], f32, tag=f"a{h}")
                    nc.vector.tensor_mul(
                        out=a[:], in0=Rt[:, 256 * h:256 * (h + 1)], in1=src)
                    mneg = wp.tile([128, 256], f32, tag=f"mneg{h}")
                    nc.vector.scalar_tensor_tensor(
                        out=mneg[:], in0=src, scalar=shift, in1=a[:],
                        op0=sub, op1=mult)
                    pT = pp.tile([128, 256], f32, tag=f"pT{h}")
                    for b in range(2):
                        nc.tensor.transpose(
                            pT[:, 128 * b:128 * (b + 1)],
                            mneg[:, 128 * b:128 * (b + 1)], s_id[:])
                    mt = wp.tile([128, 256], f32, tag=f"mTs{h}")
                    nc.scalar.copy(mt[:], pT[:])
                    mts.append(mt)

                def lhs(kc):
                    bk = kc % 4
                    off = 128 * (bk % 2) + 32 * (kc // 4)
                    return mts[bk // 2][:, off:off + 32]

                vh = []
                for h in (0, 1):
                    P1 = pp.tile([128, 256], f32, tag=f"P1h{h}")
                    for idx, kc in enumerate(ORDER):
                        lh = lhs(kc)
                        for j in range(4):
                            base = N * kc + 512 * j + 256 * h
                            nc.tensor.matmul(
                                out=P1[32 * j:32 * (j + 1), :],
                                lhsT=lh, rhs=s_Wp[:, base:base + 256],
                                start=(idx == 0), stop=(idx == 15),
                                tile_position=(0, 32 * j))
                    tmp = wp.tile([128, 256], f32, tag=f"tmp{h}")
                    nc.vector.tensor_add(
                        out=tmp[:], in0=P1[:],
                        in1=s_C[:, 256 * h:256 * (h + 1)])
                    v = wp.tile([128, 256], f32, tag=f"v{h}")
                    nc.vector.tensor_scalar(
                        out=v[:], in0=tmp[:], scalar1=MAGIC, scalar2=MAGIC,
                        op0=add, op1=sub)
                    vh.append(v)
                vh_prev = vh

            nc.sync.dma_start(out=d_out[:, 0:256], in_=vh_prev[0][:])
            nc.sync.dma_start(out=d_out[:, 256:512], in_=vh_prev[1][:])

    nc.compile()
    return nc


def _pack_w(Wneg):
    """[N, N] -> [128, 16*N]: column block kc holds rows 128*kc.. of Wneg"""
    return np.ascontiguousarray(
        Wneg.reshape(16, 128, N).transpose(1, 0, 2).reshape(128, 16 * N))


def _host_constants(r, eps, beta, K_local, W_cc, kernel_size, mode):
    """All scale folding in fp64, rounded to fp32 once."""
    pad = kernel_size // 2
    W64 = W_cc.astype(np.float64)
    C64 = np.zeros((N, N))
    idx = np.arange(N)
    for j in range(kernel_size):
        C64[(idx + j - pad) % N, idx] += np.float64(K_local[j])
    eps64 = eps.astype(np.float64)
    beta64 = beta.astype(np.float64)
    r64 = r.astype(np.float64)
    W3 = 0.5 * (1 - beta64)[None, :] * eps64[None, :] * (W64 + C64)
    W3[idx, idx] += (1 - beta64) * (1 - eps64)
    if mode == "fp32p":
        Wneg = (-127.0 * W3).astype(np.float32)
        R1 = r.astype(np.float32)
        R2 = (r64 / (127.0 * 127.0)).astype(np.float32)
        return dict(Wp=_pack_w(Wneg)), dict(R1=R1, R2=R2), beta64
    # diag-separated fp16 mode: W2[k,n] = -256*(r_k/127^2)*127*W3off[k,n]
    # so that P1 = (sq/256) @ W2 equals the off-diagonal part of
    # mapped @ (127*W3) minus the constant S (folded into C2).
    Ddiag = 127.0 * np.diag(W3).copy()         # [N]
    W3off = W3.copy()
    W3off[idx, idx] = 0.0
    Q64 = r64 / (127.0 * 127.0)
    W2 = -256.0 * 127.0 * Q64[:, None] * W3off
    wmaps = dict(Wp=_pack_w(W2.astype(np.float16)))
    S64 = 4032.25 * 127.0 * (Q64 @ W3off)                   # [N]
    Dneg64 = -Ddiag * Q64                                   # tc = D.*sq + C2
    D = Dneg64.astype(np.float32)
    C2site64 = S64 - 4032.25 * Dneg64                       # + C (per-batch) later
    return wmaps, dict(D=D, C2site=C2site64), beta64


def _to_bn(x):
    """[32, 2048] -> scrambled [128, 512]: bn[32*j + b, nt] = x[b, 512*j + nt]"""
    return np.ascontiguousarray(
        x.reshape(BL, 4, 512).transpose(1, 0, 2).reshape(128, 512))


def _from_bn(x):
    return np.ascontiguousarray(
        x.reshape(4, BL, 512).transpose(1, 0, 2).reshape(BL, N))


def _bcast_bn(site):
    """[2048] per-site constant -> scrambled [128, 512] (same for all b)"""
    return np.ascontiguousarray(np.broadcast_to(
        site.reshape(4, 1, 512), (4, BL, 512)).reshape(128, 512))


def kernel(drive, r, eps, beta, K_local, W_cc, steps=64, kernel_size=5, **_kw):
    from concourse.bass_utils import run_bass_kernel_spmd

    drive = np.asarray(drive, dtype=np.float32)
    r = np.asarray(r, dtype=np.float32)
    eps = np.asarray(eps, dtype=np.float32)
    beta = np.asarray(beta, dtype=np.float32)
    K_local = np.asarray(K_local, dtype=np.float32)
    W_cc = np.asarray(W_cc, dtype=np.float32)
    steps = int(steps)
    kernel_size = int(kernel_size)

    lo, hi = np.float32(0.0001), np.float32(1.0 - 0.0001)
    if steps <= 0:
        return np.clip(drive, lo, hi).astype(np.float32)

    wmaps, consts, beta64 = _host_constants(
        r, eps, beta, K_local, W_cc, kernel_size, MODE)
    ident = np.eye(128, dtype=np.float32)

    key = (steps, MODE)
    if key not in _programs:
        _programs[key] = _build_program(steps, mode=MODE)
    nc = _programs[key]

    in_maps = []
    for c in range(NCORES):
        dslice = drive[BL * c:BL * (c + 1)]
        C127 = 127.0 * beta64[None, :] * dslice.astype(np.float64)
        if MODE == "fp32p":
            in_maps.append(dict(
                g0_bn=_to_bn(dslice), C_bn=_to_bn(C127.astype(np.float32)),
                R1_bn=_bcast_bn(consts["R1"]), R2_bn=_bcast_bn(consts["R2"]),
                ident=ident, **wmaps))
        else:
            v0 = (127.0 * dslice).astype(np.float32)
            C2 = (C127 + consts["C2site"][None, :]).astype(np.float32)
            in_maps.append(dict(
                v0_bn=_to_bn(v0), C2_bn=_to_bn(C2),
                D_bn=_bcast_bn(consts["D"]),
                ident=ident, ident16=ident.astype(np.float16), **wmaps))

    global _last_in_maps
    _last_in_maps = in_maps
    res = run_bass_kernel_spmd(nc, in_maps, list(range(NCORES)))

    out = np.empty((B, N), dtype=np.float32)
    for c in range(NCORES):
        v = _from_bn(res.results[c]["v_out"])
        g = (v / np.float32(127.0)).astype(np.float32)
        out[BL * c:BL * (c + 1)] = np.clip(g, lo, hi)
    return out
